# revision 7
# baseline (speedup 1.0000x reference)
"""Trainium2 Bass kernel for nn_MultiHeadedAttention (B=2, H=16, S=2048, d=64).

Sharding: data-parallel over batch x tensor-parallel over heads.
8 cores = 2 batch groups x 4 head-groups (4 heads each).

v3 schedule (vs 175us baseline):
  - Inputs are HBM-bandwidth-bound (~15us for 5.6MB): host pre-arranges all
    weights into contiguous SBUF layouts, loads split across both HWDGE rings
    (sync+scalar) ordered by first use, wo cast to bf16 on host and deferred.
  - Prefix: 8 projection tiles (K/Q head-pair 0 + V st0-3) accumulate in 8
    parallel PSUM banks with the kc loop outermost, so each xT chunk is
    consumed as it lands; first scores fire ~1us after the last chunk.
  - 8 zero-matmul PE warm-ups at t~0 flip the HAM clock gate to 8/8 during
    the DMA wait; a dummy exp preloads the ACT table.
  - Priority scheme for the Tile scheduler: attention phases are emitted
    BEFORE the remaining projection/transpose/O-proj work, so the baked PE
    stream runs attention eagerly (it feeds the critical ScalarE exp stream)
    and the simulator drops filler matmuls into every exp-wait gap.
  - All four phases normalize eagerly per q-tile; transposes and O-proj are
    filler-priority, so O-proj + output DMA spread across the whole run.
  - Output shipped bf16; all PSUM->SBUF copies pinned to VectorE.
Attention math is identical to the baseline (see kernel_baseline.py).
"""

import math
from contextlib import ExitStack

import numpy as np
import ml_dtypes

import concourse.bass as bass
import concourse.mybir as mybir
import concourse.tile as tile
from concourse import bacc, bass_utils

F32 = mybir.dt.float32
BF16 = mybir.dt.bfloat16
EXP = mybir.ActivationFunctionType.Exp

B, S, D = 2, 2048, 1024
NH, HD = 16, 64
NCORES = 8
GROUPS = NCORES // B          # 4 head-groups per batch
HPC = NH // GROUPS            # 4 heads per core
M = HPC * HD                  # 256 local head-dims per core
P = 128
KC = D // P                   # 8 contraction chunks
NT = S // P                   # 16 q/s tiles
SCALE = 1.0 / math.sqrt(HD)   # 0.125
N_WARMUP = 8


def build_kernel():
    nc = bacc.Bacc("TRN2", target_bir_lowering=False)

    xT_d = nc.dram_tensor("xT", [D, S], BF16, kind="ExternalInput")
    wq_d = nc.dram_tensor("wq", [P, KC, M], BF16, kind="ExternalInput")
    wk_d = nc.dram_tensor("wk", [P, KC, M], BF16, kind="ExternalInput")
    wv_d = nc.dram_tensor("wv", [P, KC, M], BF16, kind="ExternalInput")
    wo_d = nc.dram_tensor("wo", [P, 2, D], BF16, kind="ExternalInput")
    bq_d = nc.dram_tensor("bq", [P, 2], F32, kind="ExternalInput")
    bk_d = nc.dram_tensor("bk", [P, 2], F32, kind="ExternalInput")
    tri_d = nc.dram_tensor("tri", [P, P], BF16, kind="ExternalInput")
    ident_d = nc.dram_tensor("ident", [P, P], BF16, kind="ExternalInput")
    out_d = nc.dram_tensor("out", [S, D], BF16, kind="ExternalOutput")

    with tile.TileContext(nc) as tc, ExitStack() as ctx:
        big = ctx.enter_context(tc.tile_pool(name="big", bufs=1))
        exp_pool = ctx.enter_context(tc.tile_pool(name="expp", bufs=8))
        outcp = ctx.enter_context(tc.tile_pool(name="outcp", bufs=4))
        recip_pool = ctx.enter_context(tc.tile_pool(name="recipp", bufs=2))

        # ---- persistent SBUF tiles ----
        xT_sb = big.tile([P, KC, S], BF16)
        wq_sb = big.tile([P, KC, M], BF16)
        wk_sb = big.tile([P, KC, M], BF16)
        wv_sb = big.tile([P, KC, M], BF16)
        wo_sb = big.tile([P, 2, D], BF16)
        bq_sb = big.tile([P, 2], F32)
        bk_sb = big.tile([P, 2], F32)
        qT_sb = big.tile([P, 2, S], BF16)
        kT_sb = big.tile([P, 2, S], BF16)
        v_sb = big.tile([P, NT, HPC, HD + 1], BF16)
        hs_sb = big.tile([P, NT, M], BF16)
        hsT_sb = big.tile([P, 2, NT, P], BF16)
        tri_sb = big.tile([P, P], BF16)
        ident_sb = big.tile([P, P], BF16)
        zz_sb = big.tile([1, 512], BF16)
        dummy_sb = big.tile([1, 2], BF16)

        nc.vector.memset(zz_sb[:], 0.0)
        nc.vector.memset(v_sb[:, :, :, HD : HD + 1], 1.0)
        # preload the ACT exp table (~2.7us) during the DMA prefix
        nc.scalar.activation(dummy_sb[:], zz_sb[0:1, 0:2], EXP, scale=SCALE)

        # ---- input DMAs: contiguous slabs on both rings, by first use ----
        nc.sync.dma_start(wq_sb[:], wq_d.ap())
        nc.scalar.dma_start(wk_sb[:], wk_d.ap())
        nc.scalar.dma_start(bk_sb[:], bk_d.ap())
        nc.scalar.dma_start(bq_sb[:], bq_d.ap())
        nc.scalar.dma_start(wv_sb[:], wv_d.ap())
        for kc in range(KC):
            eng = nc.sync if kc % 2 == 0 else nc.scalar
            eng.dma_start(xT_sb[:, kc, :], xT_d.ap()[P * kc : P * (kc + 1), :])
        nc.sync.dma_start(tri_sb[:], tri_d.ap())
        nc.sync.dma_start(ident_sb[:], ident_d.ap())
        nc.scalar.dma_start(wo_sb[:], wo_d.ap())

        # ---- projection helpers ----
        def qk_mm(ps, which, hp, nq, kc):
            w_sb = wq_sb if which == "q" else wk_sb
            nc.tensor.matmul(
                ps[:],
                lhsT=w_sb[:, kc, P * hp : P * (hp + 1)],
                rhs=xT_sb[:, kc, 512 * nq : 512 * (nq + 1)],
                start=(kc == 0),
                stop=(kc == KC - 1),
            )

        def qk_drain(ps, which, hp, nq):
            t_sb, b_sb = (qT_sb, bq_sb) if which == "q" else (kT_sb, bk_sb)
            nc.vector.tensor_scalar_add(
                t_sb[:, hp, 512 * nq : 512 * (nq + 1)], ps[:], b_sb[:, hp : hp + 1]
            )

        def v_mm(ps, st, kc):
            nc.tensor.matmul(
                ps[:],
                lhsT=xT_sb[:, kc, P * st : P * (st + 1)],
                rhs=wv_sb[:, kc, :],
                start=(kc == 0),
                stop=(kc == KC - 1),
            )

        def v_drain(ps, st):
            nc.vector.tensor_copy(
                v_sb[:, st, :, 0:HD], ps[:].rearrange("p (h d) -> p h d", h=HPC)
            )

        def proj_qk_tile(which, hp, nq):
            ps = fil_ps.tile([P, 512], F32, tag="fil", bufs=1, name=f"p{which}{hp}{nq}")
            for kc in range(KC):
                qk_mm(ps, which, hp, nq, kc)
            qk_drain(ps, which, hp, nq)

        def proj_v_tile(st):
            ps = fil_ps.tile([P, M], F32, tag="fil", bufs=1, name=f"pv{st}")
            for kc in range(KC):
                v_mm(ps, st, kc)
            v_drain(ps, st)

        def oproj_tile(st, dc):
            ps = fil_ps.tile([P, 512], F32, tag="fil", bufs=1, name=f"o{st}{dc}")
            for hp in range(2):
                nc.tensor.matmul(
                    ps[:],
                    lhsT=hsT_sb[:, hp, st, :],
                    rhs=wo_sb[:, hp, 512 * dc : 512 * (dc + 1)],
                    start=(hp == 0),
                    stop=(hp == 1),
                )
            o_sb = outcp.tile([P, 512], BF16, tag="o", name=f"oc{st}{dc}")
            nc.vector.tensor_copy(o_sb[:], ps[:])
            nc.sync.dma_start(
                out_d.ap()[P * st : P * (st + 1), 512 * dc : 512 * (dc + 1)], o_sb[:]
            )

        def transp_tile(hp, jq):
            # SBUF->SBUF transpose on the (idle) scalar HWDGE ring: no PE,
            # no PSUM bank, no DVE copy
            nc.scalar.dma_start(
                hsT_sb[:, hp, jq, :],
                hs_sb[:, jq, P * hp : P * (hp + 1)],
                transpose=True,
            )

        # ---- prefix: 8 tiles accumulate in 8 parallel banks, kc loop
        # outermost so each xT chunk is consumed as it arrives; PE warm-up
        # zero-MMs reuse the same tiles (reset by the kc=0 start=True) ----
        with tc.tile_pool(name="pre_ps", bufs=1, space="PSUM") as pre_ps:
            pre_spec = [("k", 0, 0), ("q", 0, 0), ("q", 0, 1), ("k", 0, 1)]
            pre_qk = [
                pre_ps.tile([P, 512], F32, tag=f"pre{i}", bufs=1, name=f"pre{i}")
                for i in range(len(pre_spec))
            ]
            pre_v = [
                pre_ps.tile([P, M], F32, tag=f"prev{st}", bufs=1, name=f"prev{st}")
                for st in range(4)
            ]
            for i in range(N_WARMUP):
                nc.tensor.matmul(
                    pre_qk[i % len(pre_qk)][:],
                    lhsT=zz_sb[0:1, 0:P],
                    rhs=zz_sb[0:1, 0:512],
                    start=True,
                    stop=True,
                    skip_group_check=True,
                )
            for kc in range(KC):
                for i, (which, hp, nq) in enumerate(pre_spec):
                    qk_mm(pre_qk[i], which, hp, nq, kc)
                for st in range(4):
                    v_mm(pre_v[st], st, kc)
            for i, (which, hp, nq) in enumerate(pre_spec):
                qk_drain(pre_qk[i], which, hp, nq)
            for st in range(4):
                v_drain(pre_v[st], st)

        attn_ps = ctx.enter_context(tc.tile_pool(name="attn_ps", bufs=1, space="PSUM"))
        fil_ps = ctx.enter_context(tc.tile_pool(name="fil_ps", bufs=1, space="PSUM"))

        # ---- attention phase (emitted attn-first = low priority) ----
        def attn_phase(hp, ph):
            qlo, qhi = 1024 * ph, 1024 * (ph + 1)
            hs_tiles = [
                attn_ps.tile([P, 455], F32, tag="hs", bufs=3, name=f"hs{hp}{ph}{i}")
                for i in range(3)
            ]

            def slot(eta, jql):
                if jql < 7:
                    return hs_tiles[eta], 65 * jql
                return hs_tiles[2], 65 * eta

            for t in hs_tiles:
                nc.tensor.matmul(
                    t[:, 0:455],
                    lhsT=zz_sb[0:1, 0:P],
                    rhs=zz_sb[0:1, 0:455],
                    start=True,
                    stop=True,
                    skip_group_check=True,
                )
            for kt in range(qhi // P):
                qstart = max(qlo, P * kt)
                for q0 in range(qstart, qhi, 512):
                    w = min(512, qhi - q0)
                    s_ps = attn_ps.tile(
                        [P, 1024], F32, tag="sc", bufs=2, name=f"sc{hp}{ph}{kt}{q0}"
                    )
                    for eta in range(2):
                        prow = slice(HD * eta, HD * (eta + 1))
                        nc.tensor.matmul(
                            s_ps[:, 512 * eta : 512 * eta + w],
                            lhsT=kT_sb[prow, hp, P * kt : P * (kt + 1)],
                            rhs=qT_sb[prow, hp, q0 : q0 + w],
                            start=True,
                            stop=True,
                        )
                    e_sb = exp_pool.tile([P, 1024], BF16, tag="e", name=f"e{kt}{q0}")
                    pair = s_ps[:].rearrange("p (g f) -> p g f", g=2)[:, :, 0:w]
                    epair = e_sb[:].rearrange("p (g f) -> p g f", g=2)[:, :, 0:w]
                    nc.scalar.activation(epair, pair, EXP, scale=SCALE)
                    if q0 == P * kt:  # chunk starts at the diagonal block
                        nc.vector.tensor_tensor(
                            e_sb[:].rearrange("p (g f) -> p g f", g=2)[:, :, 0:P],
                            e_sb[:].rearrange("p (g f) -> p g f", g=2)[:, :, 0:P],
                            tri_sb[:]
                            .rearrange("p (o f) -> p o f", o=1)
                            .broadcast_to([P, 2, P]),
                            op=mybir.AluOpType.mult,
                        )
                    for eta in range(2):
                        h = 2 * hp + eta
                        for jq in range(q0 // P, (q0 + w) // P):
                            t, col = slot(eta, jq - 8 * ph)
                            nc.tensor.matmul(
                                t[:, col : col + HD + 1],
                                lhsT=e_sb[
                                    :,
                                    512 * eta + P * jq - q0 : 512 * eta + P * jq - q0 + P,
                                ],
                                rhs=v_sb[:, kt, h, :],
                                start=False,
                                stop=(kt == jq),
                                skip_group_check=True,
                            )
                if kt >= 8 * ph:
                    # slot jq=kt complete: normalize eagerly
                    jql = kt - 8 * ph
                    recip_t = recip_pool.tile(
                        [P, 2], F32, tag="re", bufs=8, name=f"re{hp}{ph}{kt}"
                    )
                    for eta in range(2):
                        h = 2 * hp + eta
                        t, col = slot(eta, jql)
                        nc.vector.reciprocal(
                            recip_t[:, eta : eta + 1], t[:, col + HD : col + HD + 1]
                        )
                        nc.vector.tensor_scalar_mul(
                            hs_sb[:, kt, HD * h : HD * (h + 1)],
                            t[:, col : col + HD],
                            recip_t[:, eta : eta + 1],
                        )

        attn_phase(0, 0)
        attn_phase(0, 1)
        attn_phase(1, 0)
        attn_phase(1, 1)

        # ---- filler: emitted AFTER attention = higher priority number; the
        # scheduler drops these into exp-wait gaps, in this preference order,
        # gated by data deps (transp needs the eager norm, oproj needs both
        # transposes) ----
        for which, hp, nq in [("q", 0, 2), ("q", 0, 3)]:
            proj_qk_tile(which, hp, nq)
        for st in range(4, 8):
            proj_v_tile(st)
        for st in range(8, 12):
            proj_v_tile(st)
        proj_qk_tile("k", 0, 2)
        for st in range(12, 16):
            proj_v_tile(st)
        proj_qk_tile("k", 0, 3)
        for which, hp, nq in [
            ("k", 1, 0), ("q", 1, 0), ("q", 1, 1), ("k", 1, 1),
            ("q", 1, 2), ("q", 1, 3), ("k", 1, 2), ("k", 1, 3),
        ]:
            proj_qk_tile(which, hp, nq)
        for jq in range(NT):
            transp_tile(0, jq)
        for st in range(NT):
            transp_tile(1, st)
            oproj_tile(st, 0)
            oproj_tile(st, 1)

    nc.compile()
    return nc


_NC = None


def _get_nc():
    global _NC
    if _NC is None:
        _NC = build_kernel()
    return _NC


def _tri_upper(n=P):
    m = np.zeros((n, n), np.float32)
    iu = np.triu_indices(n, 0)
    m[iu] = 1.0
    return m.astype(ml_dtypes.bfloat16)


def kernel(x, W_Q, W_K, W_V, W_O, b_Q, b_K, b_V, b_O, _trace=False):
    x = np.asarray(x, np.float32)
    W_Q, W_K = np.asarray(W_Q, np.float32), np.asarray(W_K, np.float32)
    W_V, W_O = np.asarray(W_V, np.float32), np.asarray(W_O, np.float32)
    b_Q, b_K = np.asarray(b_Q, np.float32), np.asarray(b_K, np.float32)
    b_V, b_O = np.asarray(b_V, np.float32), np.asarray(b_O, np.float32)

    nc = _get_nc()
    tri = _tri_upper()
    ident = np.eye(P, dtype=np.float32).astype(ml_dtypes.bfloat16)
    xT_b = [np.ascontiguousarray(x[b].T).astype(ml_dtypes.bfloat16) for b in range(B)]

    def warr(W, cols):  # [D, Mloc] -> [P, KC, Mloc] contiguous
        return np.ascontiguousarray(
            W[:, cols].reshape(KC, P, M).transpose(1, 0, 2)
        ).astype(ml_dtypes.bfloat16)

    in_maps = []
    for core in range(NCORES):
        b, g = core // GROUPS, core % GROUPS
        cols = slice(M * g, M * (g + 1))
        in_maps.append(
            {
                "xT": xT_b[b],
                "wq": warr(W_Q, cols),
                "wk": warr(W_K, cols),
                "wv": warr(W_V, cols),
                "wo": np.ascontiguousarray(
                    W_O[cols, :].reshape(2, P, D).transpose(1, 0, 2)
                ).astype(ml_dtypes.bfloat16),
                "bq": np.ascontiguousarray(b_Q[cols].reshape(2, P).T),
                "bk": np.ascontiguousarray(b_K[cols].reshape(2, P).T),
                "tri": tri,
                "ident": ident,
            }
        )
    res = bass_utils.run_bass_kernel_spmd(
        nc, in_maps, core_ids=list(range(NCORES)), trace=_trace
    )
    const_row = (b_V @ W_O + b_O).astype(np.float32)  # exact: sum(softmax)=1
    out = np.zeros((B, S, D), np.float32)
    for b in range(B):
        acc = res.results[b * GROUPS]["out"].astype(np.float32)
        for g in range(1, GROUPS):
            acc = acc + res.results[b * GROUPS + g]["out"].astype(np.float32)
        out[b] = acc + const_row
    if _trace:
        kernel.last_results = res
    return out


# revision 13
# speedup vs baseline: 1.1385x; 1.1385x over previous
"""Trainium2 Bass kernel for nn_MultiHeadedAttention (B=2, H=16, S=2048, d=64).

Sharding: data-parallel over batch x tensor-parallel over heads.
8 cores = 2 batch groups x 4 head-groups (4 heads each).

v5 schedule (traced-informed rewrite of the 175us baseline):
  - Startup is HBM-bound (~5.6MB of inputs): weights host-pre-arranged into
    contiguous SBUF layouts, loads split across both HWDGE rings ordered by
    first use; wo bf16 + deferred.  Prefix: 8 projection tiles (K/Q hp0 +
    V st0-3) accumulate in 8 parallel PSUM banks, kc loop outermost, so each
    xT chunk is consumed on arrival.  PE warm-up zero-MMs (HAM -> 8/8) reuse
    the prefix tiles; a dummy exp preloads the ACT table.
  - The attention inner loop is ACT-bound (~1.3-1.5us per 512-chunk of exp
    vs ~0.6us of PE work), so projection/O-proj matmuls are interleaved into
    the emission at a measured per-chunk byte budget: each chunk banks
    (act_time - pe_time) and pops filler items when it can afford them.
    Filler order is chosen so every dependency (kT/qT/v tiles, hp1 weights)
    lands just before the attention phase that consumes it.
  - hs->hsT transposes ride the DMA xbar on the otherwise-idle sync ring
    (no PE, no PSUM, no DVE); output DMAs go on the scalar ring to keep the
    sync ring xbar-mode-pure.
  - V filler tiles are region-shared two-per-PSUM-bank (zero-prefill +
    start=False accumulation, same trick as the attention slots) to halve
    bank handoffs through the single filler bank.
  - Leftover O-proj drains into a 4-wide tail pool after the attention pools
    close, with copies alternating VectorE/ScalarE (ACT is free by then).
  - Output shipped bf16; in-loop PSUM->SBUF copies pinned to VectorE.
Attention math is identical to the baseline (see kernel_baseline.py):
fp32->bf16 projections, 2-head row-group-packed score matmuls, one exp per
(kt, 512-chunk) covering both heads (scale=1/8, no max subtraction),
tri-mask on diagonal blocks, PV accumulation with a ones column for the
softmax denominator, eager reciprocal+scale normalization at kt==jq.
Host adds the exact (b_V @ W_O + b_O) row.
"""

import math
from collections import deque
from contextlib import ExitStack

import numpy as np
import ml_dtypes

import concourse.bass as bass
import concourse.mybir as mybir
import concourse.tile as tile
from concourse import bacc, bass_utils

F32 = mybir.dt.float32
BF16 = mybir.dt.bfloat16
EXP = mybir.ActivationFunctionType.Exp

B, S, D = 2, 2048, 1024
NH, HD = 16, 64
NCORES = 8
GROUPS = NCORES // B          # 4 head-groups per batch
HPC = NH // GROUPS            # 4 heads per core
M = HPC * HD                  # 256 local head-dims per core
P = 128
KC = D // P                   # 8 contraction chunks
NT = S // P                   # 16 q/s tiles
SCALE = 1.0 / math.sqrt(HD)   # 0.125
N_WARMUP = 8

QK_COST = 1752                # 8 N=512 matmuls
VPAIR_COST = 2021             # prefill + 16 N=256 matmuls
OP_COST = 438                 # 2 N=512 matmuls
BUDGET_CAP = 3000.0


def build_kernel():
    nc = bacc.Bacc("TRN2", target_bir_lowering=False)

    xT_d = nc.dram_tensor("xT", [D, S], BF16, kind="ExternalInput")
    wq_d = nc.dram_tensor("wq", [P, KC, M], BF16, kind="ExternalInput")
    wk_d = nc.dram_tensor("wk", [P, KC, M], BF16, kind="ExternalInput")
    wv_d = nc.dram_tensor("wv", [P, KC, M], BF16, kind="ExternalInput")
    wo_d = nc.dram_tensor("wo", [P, 2, D], BF16, kind="ExternalInput")
    bq_d = nc.dram_tensor("bq", [P, 2], F32, kind="ExternalInput")
    bk_d = nc.dram_tensor("bk", [P, 2], F32, kind="ExternalInput")
    tri_d = nc.dram_tensor("tri", [P, P], BF16, kind="ExternalInput")
    out_d = nc.dram_tensor("out", [S, D], BF16, kind="ExternalOutput")

    with tile.TileContext(nc) as tc, ExitStack() as ctx:
        big = ctx.enter_context(tc.tile_pool(name="big", bufs=1))
        exp_pool = ctx.enter_context(tc.tile_pool(name="expp", bufs=8))
        outcp = ctx.enter_context(tc.tile_pool(name="outcp", bufs=4))
        recip_pool = ctx.enter_context(tc.tile_pool(name="recipp", bufs=2))

        # ---- persistent SBUF tiles ----
        xT_sb = big.tile([P, KC, S], BF16)
        wq_sb = big.tile([P, KC, M], BF16)
        wk_sb = big.tile([P, KC, M], BF16)
        wv_sb = big.tile([P, KC, M], BF16)
        wo_sb = big.tile([P, 2, D], BF16)
        bq_sb = big.tile([P, 2], F32)
        bk_sb = big.tile([P, 2], F32)
        qT_sb = big.tile([P, 2, S], BF16)
        kT_sb = big.tile([P, 2, S], BF16)
        v_sb = big.tile([P, NT, HPC, HD + 1], BF16)
        hs_sb = big.tile([P, NT, M], BF16)
        hsT_sb = big.tile([P, 2, NT, P], BF16)
        tri_sb = big.tile([P, P], BF16)
        zz_sb = big.tile([1, 512], BF16)
        dummy_sb = big.tile([1, 2], BF16)

        nc.vector.memset(zz_sb[:], 0.0)
        nc.vector.memset(v_sb[:, :, :, HD : HD + 1], 1.0)
        # preload the ACT exp table (~2.7us) during the DMA prefix
        nc.scalar.activation(dummy_sb[:], zz_sb[0:1, 0:2], EXP, scale=SCALE)

        # ---- input DMAs: contiguous slabs on both rings, by first use ----
        nc.sync.dma_start(wq_sb[:], wq_d.ap())
        nc.scalar.dma_start(wk_sb[:], wk_d.ap())
        nc.scalar.dma_start(bk_sb[:], bk_d.ap())
        nc.scalar.dma_start(bq_sb[:], bq_d.ap())
        nc.scalar.dma_start(wv_sb[:], wv_d.ap())
        for kc in range(KC):
            eng = nc.sync if kc % 2 == 0 else nc.scalar
            eng.dma_start(xT_sb[:, kc, :], xT_d.ap()[P * kc : P * (kc + 1), :])
        nc.sync.dma_start(tri_sb[:], tri_d.ap())
        nc.scalar.dma_start(wo_sb[:], wo_d.ap())

        # ---- projection building blocks ----
        def qk_mm(ps, which, hp, nq, kc):
            w_sb = wq_sb if which == "q" else wk_sb
            nc.tensor.matmul(
                ps[:],
                lhsT=w_sb[:, kc, P * hp : P * (hp + 1)],
                rhs=xT_sb[:, kc, 512 * nq : 512 * (nq + 1)],
                start=(kc == 0),
                stop=(kc == KC - 1),
            )

        def qk_drain(ps, which, hp, nq):
            t_sb, b_sb = (qT_sb, bq_sb) if which == "q" else (kT_sb, bk_sb)
            nc.vector.tensor_scalar_add(
                t_sb[:, hp, 512 * nq : 512 * (nq + 1)], ps[:], b_sb[:, hp : hp + 1]
            )

        def v_mm(ps, st, kc, col0=0, start=None):
            nc.tensor.matmul(
                ps[:, col0 : col0 + M],
                lhsT=xT_sb[:, kc, P * st : P * (st + 1)],
                rhs=wv_sb[:, kc, :],
                start=(kc == 0) if start is None else start,
                stop=(kc == KC - 1),
                skip_group_check=True,
            )

        def transp_tile(hp, jq):
            # SBUF->SBUF transpose on the DMA xbar via the sync ring:
            # no PE, no PSUM bank, no DVE copy
            nc.sync.dma_start(
                hsT_sb[:, hp, jq, :],
                hs_sb[:, jq, P * hp : P * (hp + 1)],
                transpose=True,
            )

        # ---- prefix: 8 tiles accumulate in 8 parallel banks, kc loop
        # outermost so each xT chunk is consumed as it arrives; PE warm-up
        # zero-MMs reuse the same tiles (reset by the kc=0 start=True) ----
        with tc.tile_pool(name="pre_ps", bufs=1, space="PSUM") as pre_ps:
            pre_spec = [("k", 0, 0), ("q", 0, 0), ("q", 0, 1), ("q", 0, 2)]
            pre_qk = [
                pre_ps.tile([P, 512], F32, tag=f"pre{i}", bufs=1, name=f"pre{i}")
                for i in range(len(pre_spec))
            ]
            pre_v = [
                pre_ps.tile([P, M], F32, tag=f"prev{st}", bufs=1, name=f"prev{st}")
                for st in range(4)
            ]
            for i in range(N_WARMUP):
                nc.tensor.matmul(
                    pre_qk[i % len(pre_qk)][:],
                    lhsT=zz_sb[0:1, 0:P],
                    rhs=zz_sb[0:1, 0:512],
                    start=True,
                    stop=True,
                    skip_group_check=True,
                )
            for kc in range(KC):
                for i, (which, hp, nq) in enumerate(pre_spec):
                    qk_mm(pre_qk[i], which, hp, nq, kc)
                for st in range(4):
                    v_mm(pre_v[st], st, kc)
            for i, (which, hp, nq) in enumerate(pre_spec):
                qk_drain(pre_qk[i], which, hp, nq)
            for st in range(4):
                nc.vector.tensor_copy(
                    v_sb[:, st, :, 0:HD],
                    pre_v[st][:].rearrange("p (h d) -> p h d", h=HPC),
                )

        # ---- main pools: hs 3 + sc 4 + fil 1 = 8 banks ----
        with tc.tile_pool(name="attn_ps", bufs=1, space="PSUM") as attn_ps, \
             tc.tile_pool(name="fil_ps", bufs=1, space="PSUM") as fil_ps:

            # filler items: (pe_cost_ns, closure); emitted when afforded
            filler = deque()
            state = {"budget": 0.0}

            def mk_qk(which, hp, nq):
                def fn():
                    ps = fil_ps.tile(
                        [P, 512], F32, tag="fil", bufs=1, name=f"p{which}{hp}{nq}"
                    )
                    for kc in range(KC):
                        qk_mm(ps, which, hp, nq, kc)
                    qk_drain(ps, which, hp, nq)
                return QK_COST, fn, None

            def mk_vpair(st):
                def fn():
                    ps = fil_ps.tile([P, 512], F32, tag="fil", bufs=1, name=f"pv{st}")
                    nc.tensor.matmul(
                        ps[:],
                        lhsT=zz_sb[0:1, 0:P],
                        rhs=zz_sb[0:1, 0:512],
                        start=True,
                        stop=True,
                        skip_group_check=True,
                    )
                    for kc in range(KC):
                        v_mm(ps, st, kc, col0=0, start=False)
                        v_mm(ps, st + 1, kc, col0=M, start=False)
                    nc.vector.tensor_copy(
                        v_sb[:, st : st + 2, :, 0:HD],
                        ps[:].rearrange("p (s h d) -> p s h d", s=2, h=HPC),
                    )
                return VPAIR_COST, fn, None

            def emit_oproj(pool, tag, bufs, st, dc, copy_eng="v"):
                ps = pool.tile([P, 512], F32, tag=tag, bufs=bufs, name=f"o{st}{dc}")
                for hp in range(2):
                    nc.tensor.matmul(
                        ps[:],
                        lhsT=hsT_sb[:, hp, st, :],
                        rhs=wo_sb[:, hp, 512 * dc : 512 * (dc + 1)],
                        start=(hp == 0),
                        stop=(hp == 1),
                    )
                o_sb = outcp.tile([P, 512], BF16, tag="o", name=f"oc{st}{dc}")
                if copy_eng == "s":
                    nc.scalar.copy(o_sb[:], ps[:])
                else:
                    nc.vector.tensor_copy(o_sb[:], ps[:])
                nc.scalar.dma_start(
                    out_d.ap()[P * st : P * (st + 1), 512 * dc : 512 * (dc + 1)],
                    o_sb[:],
                )

            def mk_oproj(st, dc):
                def fn():
                    emit_oproj(fil_ps, "fil", 1, st, dc)
                return OP_COST, fn, ("op", st, dc)

            def drain_filler():
                while filler and state["budget"] >= filler[0][0]:
                    item = filler.popleft()
                    item[1]()
                    state["budget"] -= item[0]

            def add_budget(ns):
                state["budget"] = min(state["budget"] + ns, BUDGET_CAP)
                drain_filler()

            filler.extend(
                [
                    mk_qk("k", 0, 1),
                    mk_vpair(4),
                    mk_qk("q", 0, 3),
                    mk_vpair(6),
                    mk_qk("k", 1, 0),
                    mk_qk("q", 1, 0),
                    mk_qk("q", 1, 1),
                    mk_qk("k", 0, 2),
                    mk_vpair(8),
                    mk_qk("k", 1, 1),
                    mk_vpair(10),
                    mk_vpair(12),
                    mk_qk("k", 0, 3),
                    mk_vpair(14),
                    mk_qk("q", 1, 2),
                    mk_qk("q", 1, 3),
                    mk_qk("k", 1, 2),
                    mk_qk("k", 1, 3),
                ]
            )

            def attn_phase(hp, ph):
                qlo, qhi = 1024 * ph, 1024 * (ph + 1)
                hs_tiles = [
                    attn_ps.tile([P, 455], F32, tag="hs", bufs=3, name=f"hs{hp}{ph}{i}")
                    for i in range(3)
                ]

                def slot(eta, jql):
                    if jql < 7:
                        return hs_tiles[eta], 65 * jql
                    return hs_tiles[2], 65 * eta

                for t in hs_tiles:
                    nc.tensor.matmul(
                        t[:, 0:455],
                        lhsT=zz_sb[0:1, 0:P],
                        rhs=zz_sb[0:1, 0:455],
                        start=True,
                        stop=True,
                        skip_group_check=True,
                    )
                for kt in range(qhi // P):
                    qstart = max(qlo, P * kt)
                    for q0 in range(qstart, qhi, 512):
                        w = min(512, qhi - q0)
                        s_ps = attn_ps.tile(
                            [P, 1024], F32, tag="sc", bufs=2, name=f"sc{hp}{ph}{kt}{q0}"
                        )
                        for eta in range(2):
                            prow = slice(HD * eta, HD * (eta + 1))
                            nc.tensor.matmul(
                                s_ps[:, 512 * eta : 512 * eta + w],
                                lhsT=kT_sb[prow, hp, P * kt : P * (kt + 1)],
                                rhs=qT_sb[prow, hp, q0 : q0 + w],
                                start=True,
                                stop=True,
                            )
                        e_sb = exp_pool.tile(
                            [P, 1024], BF16, tag="e", name=f"e{kt}{q0}"
                        )
                        pair = s_ps[:].rearrange("p (g f) -> p g f", g=2)[:, :, 0:w]
                        epair = e_sb[:].rearrange("p (g f) -> p g f", g=2)[:, :, 0:w]
                        nc.scalar.activation(epair, pair, EXP, scale=SCALE)
                        if q0 == P * kt:  # chunk starts at the diagonal block
                            nc.vector.tensor_tensor(
                                e_sb[:].rearrange("p (g f) -> p g f", g=2)[:, :, 0:P],
                                e_sb[:].rearrange("p (g f) -> p g f", g=2)[:, :, 0:P],
                                tri_sb[:]
                                .rearrange("p (o f) -> p o f", o=1)
                                .broadcast_to([P, 2, P]),
                                op=mybir.AluOpType.mult,
                            )
                        nblk = 0
                        for eta in range(2):
                            h = 2 * hp + eta
                            for jq in range(q0 // P, (q0 + w) // P):
                                t, col = slot(eta, jq - 8 * ph)
                                nc.tensor.matmul(
                                    t[:, col : col + HD + 1],
                                    lhsT=e_sb[
                                        :,
                                        512 * eta + P * jq - q0 :
                                        512 * eta + P * jq - q0 + P,
                                    ],
                                    rhs=v_sb[:, kt, h, :],
                                    start=False,
                                    stop=(kt == jq),
                                    skip_group_check=True,
                                )
                                nblk += 1
                        act_ns = (2 * w + 352) / 1.2 + 330
                        pe_ns = w / 2.4 + nblk * 45
                        add_budget(act_ns - pe_ns)
                    if kt >= 8 * ph:
                        # slot jq=kt complete: normalize, transpose (DMA)
                        jql = kt - 8 * ph
                        recip_t = recip_pool.tile(
                            [P, 2], F32, tag="re", bufs=8, name=f"re{hp}{ph}{kt}"
                        )
                        for eta in range(2):
                            h = 2 * hp + eta
                            t, col = slot(eta, jql)
                            nc.vector.reciprocal(
                                recip_t[:, eta : eta + 1],
                                t[:, col + HD : col + HD + 1],
                            )
                            nc.vector.tensor_scalar_mul(
                                hs_sb[:, kt, HD * h : HD * (h + 1)],
                                t[:, col : col + HD],
                                recip_t[:, eta : eta + 1],
                            )
                        transp_tile(hp, kt)
                        if hp == 1:
                            filler.append(mk_oproj(kt, 0))
                            filler.append(mk_oproj(kt, 1))

            attn_phase(0, 0)
            attn_phase(0, 1)
            attn_phase(1, 0)
            attn_phase(1, 1)
            leftovers = list(filler)
            filler.clear()

            # tail: leftover O-proj through 4 parallel fil-tagged banks is
            # not possible (fil bufs=1), so re-emit leftovers into a wide
            # tail pool AFTER this pool closes (handled below via specs)
            tail_specs = []
            for item in leftovers:
                if item[2] is not None:
                    tail_specs.append(item[2])
                else:
                    item[1]()  # stray projection tile: emit now

        # ---- tail: leftover O-proj through 4 parallel banks; ACT is free
        # now so copies alternate VectorE/ScalarE ----
        if tail_specs:
            with tc.tile_pool(name="tail_ps", bufs=1, space="PSUM") as tail_ps:
                for idx, (_, st, dc) in enumerate(tail_specs):
                    emit_oproj(
                        tail_ps, "t", 4, st, dc, copy_eng="s" if idx % 2 else "v"
                    )

    nc.compile()
    return nc


_NC = None


def _get_nc():
    global _NC
    if _NC is None:
        _NC = build_kernel()
    return _NC


def _tri_upper(n=P):
    m = np.zeros((n, n), np.float32)
    iu = np.triu_indices(n, 0)
    m[iu] = 1.0
    return m.astype(ml_dtypes.bfloat16)


def kernel(x, W_Q, W_K, W_V, W_O, b_Q, b_K, b_V, b_O, _trace=False):
    x = np.asarray(x, np.float32)
    W_Q, W_K = np.asarray(W_Q, np.float32), np.asarray(W_K, np.float32)
    W_V, W_O = np.asarray(W_V, np.float32), np.asarray(W_O, np.float32)
    b_Q, b_K = np.asarray(b_Q, np.float32), np.asarray(b_K, np.float32)
    b_V, b_O = np.asarray(b_V, np.float32), np.asarray(b_O, np.float32)

    nc = _get_nc()
    tri = _tri_upper()
    xT_b = [np.ascontiguousarray(x[b].T).astype(ml_dtypes.bfloat16) for b in range(B)]

    def warr(W, cols):  # [D, Mloc] -> [P, KC, Mloc] contiguous
        return np.ascontiguousarray(
            W[:, cols].reshape(KC, P, M).transpose(1, 0, 2)
        ).astype(ml_dtypes.bfloat16)

    in_maps = []
    for core in range(NCORES):
        b, g = core // GROUPS, core % GROUPS
        cols = slice(M * g, M * (g + 1))
        in_maps.append(
            {
                "xT": xT_b[b],
                "wq": warr(W_Q, cols),
                "wk": warr(W_K, cols),
                "wv": warr(W_V, cols),
                "wo": np.ascontiguousarray(
                    W_O[cols, :].reshape(2, P, D).transpose(1, 0, 2)
                ).astype(ml_dtypes.bfloat16),
                "bq": np.ascontiguousarray(b_Q[cols].reshape(2, P).T),
                "bk": np.ascontiguousarray(b_K[cols].reshape(2, P).T),
                "tri": tri,
            }
        )
    res = bass_utils.run_bass_kernel_spmd(
        nc, in_maps, core_ids=list(range(NCORES)), trace=_trace
    )
    const_row = (b_V @ W_O + b_O).astype(np.float32)  # exact: sum(softmax)=1
    out = np.zeros((B, S, D), np.float32)
    for b in range(B):
        acc = res.results[b * GROUPS]["out"].astype(np.float32)
        for g in range(1, GROUPS):
            acc = acc + res.results[b * GROUPS + g]["out"].astype(np.float32)
        out[b] = acc + const_row
    if _trace:
        kernel.last_results = res
    return out
